# revision 1
# baseline (speedup 1.0000x reference)
"""Cosine-similarity clustering layer (retrieval kNN) on 8 Trainium2 cores.

Computes sim = ((x/|x|) @ (c/|c|).T + 1) / 2 for x [64,512,1024], c [256,1024].

Strategy: data-parallel over the 32768 flattened rows of x (4096 rows per
core), cluster centers replicated. Per core:
  - centers: fp32 norms on ScalarE (Square + accum), scale+cast to fp16 in one
    activation, XBAR DMA-transpose to [d, c] layout (contraction on partitions)
  - x streams in 0.5 MB SWDGE DMAs that cast fp32->fp16 in flight (no engine
    cycles spent on the cast)
  - row norms: ScalarE Square + accum_out per 128-row tile, then
    sqrt(4*ss) = 2|x| on ScalarE and a VectorE reciprocal -> 0.5/|x|
  - transpose to [d, m]: even tiles via PE is_transpose matmuls into a PSUM
    bank + one fused [128,1024] PSUM->SBUF copy on VectorE; odd tiles via a
    single SBUF->SBUF XBAR DMA-transpose (split paths balance PE vs DMA)
  - GEMM: 8 accumulating fp16 matmuls into PSUM [128,256] (fp32 accumulate)
  - epilogue folds the x-norm and (s+1)/2 into one tensor_scalar:
    out = psum * (0.5/|x_row|) + 0.5, then 2-tile batched stores.
Norm scaling happens after the GEMM (32768x256 elements) instead of
normalizing x itself (32768x1024) - 4x less elementwise work. Each m-tile
gets a dedicated xT slot (bufs=32) so transposes never wait on GEMM drain.
"""

import sys

import numpy as np

for _p in ("/opt/trn_rl_repo",):
    if _p not in sys.path:
        sys.path.insert(0, _p)

N_CORES = 8
B, S, D = 64, 512, 1024
K = 256                      # n_clusters
ROWS = (B * S) // N_CORES    # 4096 rows per core
P = 128
MT = ROWS // P               # 32 m-tiles per core
DCH = D // P                 # 8 contraction chunks
KT = K // P                  # 2 center tiles
GROUP = 4                    # m-tiles per load batch

_cache = {}


def build_module():
    import concourse.bacc as bacc
    import concourse.mybir as mybir
    import concourse.tile as tile
    from concourse.masks import make_identity

    f32 = mybir.dt.float32
    f16 = mybir.dt.float16
    Act = mybir.ActivationFunctionType
    Alu = mybir.AluOpType

    nc = bacc.Bacc("TRN2", target_bir_lowering=False, debug=False)
    x = nc.dram_tensor("x", [ROWS, D], f32, kind="ExternalInput")
    c = nc.dram_tensor("c", [K, D], f32, kind="ExternalInput")
    out = nc.dram_tensor("out", [ROWS, K], f32, kind="ExternalOutput")

    with tile.TileContext(nc) as tc:
        with (
            tc.tile_pool(name="const", bufs=1) as cpool,
            tc.tile_pool(name="xload", bufs=6) as xpool,
            tc.tile_pool(name="work", bufs=3) as wpool,
            tc.tile_pool(name="xtp", bufs=32) as xtpool,
            tc.tile_pool(name="norms", bufs=8) as npool,
            tc.tile_pool(name="outp", bufs=4) as opool,
            tc.tile_pool(name="psum_t", bufs=3, space="PSUM") as ptpool,
            tc.tile_pool(name="psum_mm", bufs=5, space="PSUM") as ppool,
        ):
            ident = cpool.tile([P, P], f16, name="ident")
            make_identity(nc, ident[:])

            # ---- centers: fp32 norms, scale+cast to fp16, transpose ----
            cnT = cpool.tile([P, DCH, K], f16, name="cnT")
            css = cpool.tile([P, KT], f32, name="css")
            cf_tiles = []
            for i in range(KT):
                cf = cpool.tile([P, D], f32, name=f"cf{i}")
                nc.sync.dma_start(cf[:], c[i * P : (i + 1) * P, :])
                csq = cpool.tile([P, D], f32, name="csq")
                nc.scalar.activation(
                    csq[:], cf[:], Act.Square, accum_out=css[:, i : i + 1]
                )
                cf_tiles.append(cf)
            # rc = 1/|c| (norms ~32 for randn rows; eps clamp unreachable)
            cnorm = cpool.tile([P, KT], f32, name="cnorm")
            rc = cpool.tile([P, KT], f32, name="rc")
            nc.scalar.activation(cnorm[:], css[:], Act.Sqrt)
            nc.vector.reciprocal(rc[:], cnorm[:])
            for i in range(KT):
                cb = cpool.tile([P, D], f16, name=f"cb{i}")
                nc.scalar.activation(
                    cb[:], cf_tiles[i][:], Act.Copy, scale=rc[:, i : i + 1]
                )
                # one XBAR transpose per center tile:
                # cnT[p, o, i*128+m] = cb[m, o*128+p]
                nc.sync.dma_start_transpose(
                    cnT[:, :, i * P : (i + 1) * P], cb[:]
                )

            # ---- x tiles, in groups of GROUP m-tiles ----
            for g in range(MT // GROUP):
                r0 = g * GROUP * P
                xb16 = xpool.tile([P, GROUP, D], f16, name="xb16")
                nc.gpsimd.dma_start(
                    xb16[:],
                    x[r0 : r0 + GROUP * P, :].rearrange("(n p) d -> p n d", p=P),
                )
                obat = None
                for i in range(GROUP):
                    t = g * GROUP + i
                    x16 = xb16[:, i, :]
                    ss = npool.tile([P, 1], f32, name="ss")
                    sqt = wpool.tile([P, D], f16, name="sqt")
                    nc.scalar.activation(
                        sqt[:], x16, Act.Square, accum_out=ss[:]
                    )
                    # rnh = 0.5/|x_row|: sqrt(4*ss) = 2|x|, then reciprocal
                    rnh = npool.tile([P, 1], f32, name="rnh")
                    nc.scalar.activation(rnh[:], ss[:], Act.Sqrt, scale=4.0)
                    nc.vector.reciprocal(rnh[:], rnh[:])
                    xT = xtpool.tile([P, DCH, P], f16, name="xT")
                    if t % 2 == 0:
                        # PE transpose into a PSUM bank + one fused copy out
                        psT = ptpool.tile([P, DCH, P], f16, name="psT")
                        for j in range(DCH):
                            nc.tensor.transpose(
                                psT[:, j, :], x16[:, j * P : (j + 1) * P], ident[:]
                            )
                        nc.vector.tensor_copy(xT[:], psT[:])
                    else:
                        # XBAR transpose SBUF->SBUF, all 8 chunks in one DMA:
                        # xT[p, o, m] = x16[m, o*128+p]
                        nc.sync.dma_start_transpose(xT[:], x16)
                    ps = ppool.tile([P, K], f32, name="ps")
                    for j in range(DCH):
                        nc.tensor.matmul(
                            ps[:],
                            xT[:, j, :],
                            cnT[:, j, :],
                            start=(j == 0),
                            stop=(j == DCH - 1),
                        )
                    if i % 2 == 0:
                        obat = opool.tile([P, 2, K], f32, name="obat")
                    nc.vector.tensor_scalar(
                        obat[:, i % 2, :],
                        ps[:],
                        rnh[:],
                        0.5,
                        Alu.mult,
                        Alu.add,
                    )
                    if i % 2 == 1:
                        rr = r0 + (i - 1) * P
                        nc.sync.dma_start(
                            out[rr : rr + 2 * P, :].rearrange(
                                "(n p) k -> p n k", p=P
                            ),
                            obat[:],
                        )
    nc.compile()
    return nc


def get_module():
    if "nc" not in _cache:
        _cache["nc"] = build_module()
    return _cache["nc"]


def kernel(x, cluster_centers):
    from concourse.bass_utils import run_bass_kernel_spmd

    x = np.ascontiguousarray(np.asarray(x, dtype=np.float32))
    c = np.ascontiguousarray(np.asarray(cluster_centers, dtype=np.float32))
    b, s, d = x.shape
    xf = x.reshape(-1, d)
    shards = np.split(xf, N_CORES, axis=0)
    nc = get_module()
    in_maps = [{"x": np.ascontiguousarray(sh), "c": c} for sh in shards]
    res = run_bass_kernel_spmd(nc, in_maps, list(range(N_CORES)))
    outs = [np.asarray(res.results[i]["out"]) for i in range(N_CORES)]
    return np.concatenate(outs, axis=0).reshape(b, s, K)



# revision 3
# speedup vs baseline: 1.8414x; 1.8414x over previous
"""Cosine-similarity clustering layer (retrieval kNN) on 8 Trainium2 cores.

Computes sim = ((x/|x|) @ (c/|c|).T + 1) / 2 for x [64,512,1024], c [256,1024].

Strategy: data-parallel over the 32768 flattened rows of x (4096 rows per
core), cluster centers replicated. Per core:
  - centers: fp32 norms on ScalarE (Square + accum), scale+cast to fp16 in one
    activation, XBAR DMA-transpose to [d, c] layout (contraction on partitions)
  - x streams in 0.5 MB SWDGE DMAs that cast fp32->fp16 in flight (no engine
    cycles spent on the cast)
  - row norms: ScalarE Square + accum_out per 128-row tile, then
    sqrt(4*ss) = 2|x| on ScalarE and a VectorE reciprocal -> 0.5/|x|
  - transpose to [d, m]: even tiles via PE is_transpose matmuls into a PSUM
    bank + one fused [128,1024] PSUM->SBUF copy on VectorE; odd tiles via a
    single SBUF->SBUF XBAR DMA-transpose (split paths balance PE vs DMA)
  - GEMM: 8 accumulating fp16 matmuls into PSUM [128,256] (fp32 accumulate)
  - epilogue folds the x-norm and (s+1)/2 into one tensor_scalar:
    out = psum * (0.5/|x_row|) + 0.5, then 2-tile batched stores.
Norm scaling happens after the GEMM (32768x256 elements) instead of
normalizing x itself (32768x1024) - 4x less elementwise work. Each m-tile
gets a dedicated xT slot (bufs=32) so transposes never wait on GEMM drain.
"""

import sys

import numpy as np

for _p in ("/opt/trn_rl_repo",):
    if _p not in sys.path:
        sys.path.insert(0, _p)

N_CORES = 8
B, S, D = 64, 512, 1024
K = 256                      # n_clusters
ROWS = (B * S) // N_CORES    # 4096 rows per core
P = 128
MT = ROWS // P               # 32 m-tiles per core
DCH = D // P                 # 8 contraction chunks
KT = K // P                  # 2 center tiles
GROUP = 4                    # m-tiles per load batch

_cache = {}


def build_module():
    import concourse.bacc as bacc
    import concourse.mybir as mybir
    import concourse.tile as tile
    from concourse.masks import make_identity

    f32 = mybir.dt.float32
    f16 = mybir.dt.float16
    Act = mybir.ActivationFunctionType
    Alu = mybir.AluOpType

    nc = bacc.Bacc("TRN2", target_bir_lowering=False, debug=False)
    x = nc.dram_tensor("x", [ROWS, D], f32, kind="ExternalInput")
    c = nc.dram_tensor("c", [K, D], f32, kind="ExternalInput")
    out = nc.dram_tensor("out", [ROWS, K], f32, kind="ExternalOutput")

    with tile.TileContext(nc) as tc:
        with (
            tc.tile_pool(name="const", bufs=1) as cpool,
            tc.tile_pool(name="xload", bufs=6) as xpool,
            tc.tile_pool(name="work", bufs=3) as wpool,
            tc.tile_pool(name="xtp", bufs=32) as xtpool,
            tc.tile_pool(name="norms", bufs=8) as npool,
            tc.tile_pool(name="outp", bufs=4) as opool,
            tc.tile_pool(name="psum_t", bufs=3, space="PSUM") as ptpool,
            tc.tile_pool(name="psum_mm", bufs=5, space="PSUM") as ppool,
        ):
            ident = cpool.tile([P, P], f16, name="ident")
            make_identity(nc, ident[:])

            # ---- centers: fp32 norms, scale+cast to fp16, transpose ----
            cnT = cpool.tile([P, DCH, K], f16, name="cnT")
            css = cpool.tile([P, KT], f32, name="css")
            cf_tiles = []
            for i in range(KT):
                cf = cpool.tile([P, D], f32, name=f"cf{i}")
                nc.sync.dma_start(cf[:], c[i * P : (i + 1) * P, :])
                csq = cpool.tile([P, D], f32, name="csq")
                nc.scalar.activation(
                    csq[:], cf[:], Act.Square, accum_out=css[:, i : i + 1]
                )
                cf_tiles.append(cf)
            # rc = 1/|c| (norms ~32 for randn rows; eps clamp unreachable)
            cnorm = cpool.tile([P, KT], f32, name="cnorm")
            rc = cpool.tile([P, KT], f32, name="rc")
            nc.scalar.activation(cnorm[:], css[:], Act.Sqrt)
            nc.vector.reciprocal(rc[:], cnorm[:])
            for i in range(KT):
                cb = cpool.tile([P, D], f16, name=f"cb{i}")
                nc.scalar.activation(
                    cb[:], cf_tiles[i][:], Act.Copy, scale=rc[:, i : i + 1]
                )
                # one XBAR transpose per center tile:
                # cnT[p, o, i*128+m] = cb[m, o*128+p]
                nc.sync.dma_start_transpose(
                    cnT[:, :, i * P : (i + 1) * P], cb[:]
                )

            # ---- x tiles, in groups of GROUP m-tiles ----
            for g in range(MT // GROUP):
                r0 = g * GROUP * P
                xb16 = xpool.tile([P, GROUP, D], f16, name="xb16")
                nc.gpsimd.dma_start(
                    xb16[:],
                    x[r0 : r0 + GROUP * P, :].rearrange("(n p) d -> p n d", p=P),
                )
                obat = None
                for i in range(GROUP):
                    t = g * GROUP + i
                    x16 = xb16[:, i, :]
                    ss = npool.tile([P, 1], f32, name="ss")
                    sqt = wpool.tile([P, D], f16, name="sqt")
                    nc.scalar.activation(
                        sqt[:], x16, Act.Square, accum_out=ss[:]
                    )
                    # rnh = 0.5/|x_row|: sqrt(4*ss) = 2|x|, then reciprocal
                    rnh = npool.tile([P, 1], f32, name="rnh")
                    nc.scalar.activation(rnh[:], ss[:], Act.Sqrt, scale=4.0)
                    nc.vector.reciprocal(rnh[:], rnh[:])
                    xT = xtpool.tile([P, DCH, P], f16, name="xT")
                    if t % 2 == 0:
                        # PE transpose into a PSUM bank + one fused copy out
                        psT = ptpool.tile([P, DCH, P], f16, name="psT")
                        for j in range(DCH):
                            nc.tensor.transpose(
                                psT[:, j, :], x16[:, j * P : (j + 1) * P], ident[:]
                            )
                        nc.vector.tensor_copy(xT[:], psT[:])
                    else:
                        # XBAR transpose SBUF->SBUF, all 8 chunks in one DMA:
                        # xT[p, o, m] = x16[m, o*128+p]
                        nc.sync.dma_start_transpose(xT[:], x16)
                    ps = ppool.tile([P, K], f32, name="ps")
                    for j in range(DCH):
                        nc.tensor.matmul(
                            ps[:],
                            xT[:, j, :],
                            cnT[:, j, :],
                            start=(j == 0),
                            stop=(j == DCH - 1),
                        )
                    if i % 2 == 0:
                        obat = opool.tile([P, 2, K], f32, name="obat")
                    nc.vector.tensor_scalar(
                        obat[:, i % 2, :],
                        ps[:],
                        rnh[:],
                        0.5,
                        Alu.mult,
                        Alu.add,
                    )
                    if i % 2 == 1:
                        rr = r0 + (i - 1) * P
                        nc.sync.dma_start(
                            out[rr : rr + 2 * P, :].rearrange(
                                "(n p) k -> p n k", p=P
                            ),
                            obat[:],
                        )
    nc.compile()
    return nc


def get_module():
    if "nc" not in _cache:
        _cache["nc"] = build_module()
    return _cache["nc"]


OUT_NAMES = ["out"]


def shard_inputs(x2d, c):
    shards = np.split(x2d, N_CORES, axis=0)
    return [
        {"x": np.ascontiguousarray(s), "c": np.ascontiguousarray(c)}
        for s in shards
    ]


def unshard_core0(outs):
    return outs["out"]


def unshard_full(outs):
    # outs: name -> [n_cores, *core_shape]
    return outs["out"].reshape(-1, K)


def kernel(x, cluster_centers):
    from concourse.bass_utils import run_bass_kernel_spmd

    x = np.ascontiguousarray(np.asarray(x, dtype=np.float32))
    c = np.ascontiguousarray(np.asarray(cluster_centers, dtype=np.float32))
    b, s, d = x.shape
    xf = x.reshape(-1, d)
    nc = get_module()
    in_maps = shard_inputs(xf, c)
    res = run_bass_kernel_spmd(nc, in_maps, list(range(N_CORES)))
    outs = [np.asarray(res.results[i]["out"]) for i in range(N_CORES)]
    return np.concatenate(outs, axis=0).reshape(b, s, K)

